# revision 26
# baseline (speedup 1.0000x reference)
"""Fused multi-LoRA linear layer on 8 TRN2 NeuronCores.

out = x @ W.T + b + scale * mask(x @ A_all^T) @ B_flat

Sharding: data-parallel over the token dim N (32768 -> 8 x 4096).
Weights (W, A_all, B_all, b) are replicated; each core computes its token
shard fully, so no collectives are needed.

v5: wider fp8 coverage tuned to both the error budget and the chip's
power-state ceiling, plus LDWEIGHTS pair-sharing and ring balancing.
- 13 of 16 output row-groups run 3 fp8e4 DoubleRow k-pairs (contraction
  dims 0..767); the rest use 2 pairs. Host-side bit-exact simulation puts
  rel err at 1.934e-2 (sim matches HW to ~6 digits) under the 2e-2 gate.
- IMPORTANT: pushing to 14 KQ=3 groups (432 DR matmuls/core vs 424) trips
  a power-profile downclock -- the PE drops from 2.4 to ~2.0 GHz for the
  whole run (all matmuls 216 -> 259 ns) and the kernel LOSES ~80us. The
  DR-matmul density ceiling binds before the accuracy budget does.
- Down-projection and fp8 open matmuls of a chunk pair interleave so each
  256-col DR LDWEIGHTS serves two matmuls (halves DR weight-load stalls).
- Output is written bf16 (halves output HBM traffic; +3e-6 error var) and
  output DMAs alternate between the sync and scalar HWDGE rings; the last
  chunk's tail DMAs split across both rings.
- Startup: warm-tile memset on GpSimd, warmup+filler matmuls bridge the
  chunk-0 DMA ramp so HAM never re-throttles; chunk-1 loads are split so
  its down-projection starts on first-piece arrival.
"""

import numpy as np
import ml_dtypes

# Problem constants (hardcoded per harness contract).
N, D_IN, D_OUT, L, R = 32768, 2048, 2048, 8, 16
SCALE = 32.0 / 16.0
M_CORES = 8
NS = N // M_CORES  # 4096 tokens per core
P = 128
KT = D_IN // P  # 16 k-tiles
KQMAX = 3  # max fp8 DoubleRow k-PAIRS in the base matmul
N3 = 13  # output row-groups (of OI) using KQMAX pairs; the rest use 2
KB = KT - 4  # bf16 k-tile slots (k-tiles 4..15); KQ=3 groups skip the first 2
KD = KT // 2  # fp8 k-pairs in the down-projection (all 8)
OI = D_OUT // P  # 16 output row-chunks of 128
TW = 512  # token tile width (moving free dim)
TC = NS // TW  # 8 token chunks per core
LR = L * R  # 128
WG = 4  # W column groups
WGC = D_OUT // WG  # 512 columns per group
WARM = 112  # PE warmup matmuls before the first real matmul
FILL = 96  # startup filler matmuls (bridge chunk-0 DMA window)
PAIR_ILV = True  # interleave chunk-pair DR matmuls (share LDWEIGHTS)
GATE_C1 = True  # gate chunk-1 loads behind quad-0 completion
GATE_LATE = True  # gate bF/g1..3 slabs behind the first down matmul
OUT_SPLIT = True  # alternate output DMAs between sync/scalar rings
OUT_BF16 = True  # write outT in bf16 (halves output HBM traffic)
TRIM_WT = True  # skip DMA of never-read bf16 weight slots
SWDGE_Q = 1  # GpSimd software-DGE queues
VEC_DRAIN = False  # vector-engine PSUM drain: measured ~1-2us slower
SOLO_SPACED = False  # spaced solo-chunk opens: measured ~1us slower

SX = 32.0  # fp8 scale for x
SQ = 2048.0  # fp8 scale for W and A
GS = SX * SQ  # 2^16: global scale of all device-side math

_BF16 = ml_dtypes.bfloat16
_F8 = ml_dtypes.float8_e4m3

_CACHE = {}

LAST_EXEC_TIME_NS = None


def _kq_of(oi):
    return KQMAX if oi < N3 else 2


def _kb0_of(oi):
    # first usable bf16 k-tile slot (slot kb holds k-tile kb+4)
    return 2 if oi < N3 else 0


def _build():
    import concourse.bass as bass  # noqa: F401
    import concourse.tile as tile
    from concourse import bacc, mybir
    from concourse.tile_rust import add_dep_helper
    from contextlib import ExitStack

    bf16 = mybir.dt.bfloat16
    f8 = mybir.dt.float8e4
    f32 = mybir.dt.float32
    DR = mybir.MatmulPerfMode.DoubleRow

    nc = bacc.Bacc(
        "TRN2",
        target_bir_lowering=False,
        debug=False,
        num_devices=M_CORES,
        num_swdge_queues=SWDGE_Q,
    )

    # Host-prepared, partition-major layouts (see kernel()):
    #   xT [TC, P, KB, TW] bf16 : xT[t,p,kb,j] = x[t*TW+j, (kb+4)*P+p]
    #   xQ [TC, P, KD, 2, TW] f8: xQ[t,p,kk,u,j] = q8(x[t*TW+j, (2kk+u)*P+p]*SX)
    #   wT [WG, P, KB, WGC] bf16: wT[g,p,kb,o] = W[g*WGC+o, (kb+4)*P+p]*GS
    #   wQ [P, KQMAX, 2, D_OUT] f8: wQ[p,kk,u,o] = q8(W[o, (2kk+u)*P+p]*SQ)
    #   aQ [P, KD, 2, LR] f8    : aQ[p,kk,u,c] = q8(A_flat[c, (2kk+u)*P+p]*SQ)
    #   bF [P, D_OUT] bf16      : bF[c,o] = B_all[c//R, o, c%R]
    #   bias [P, OI] f32        : bias[p,oi] = b[oi*P+p]*GS
    #   mT [TC, P, TW] bf16     : one-hot adapter mask * SCALE
    xT = nc.dram_tensor("xT", [TC, P, KB, TW], bf16, kind="ExternalInput").ap()
    xQ = nc.dram_tensor("xQ", [TC, P, KD, 2, TW], f8, kind="ExternalInput").ap()
    wT = nc.dram_tensor("wT", [WG, P, KB, WGC], bf16, kind="ExternalInput").ap()
    wQ = nc.dram_tensor("wQ", [P, KQMAX, 2, D_OUT], f8, kind="ExternalInput").ap()
    aQ = nc.dram_tensor("aQ", [P, KD, 2, LR], f8, kind="ExternalInput").ap()
    bF = nc.dram_tensor("bF", [P, D_OUT], bf16, kind="ExternalInput").ap()
    bias = nc.dram_tensor("bias", [P, OI], f32, kind="ExternalInput").ap()
    mT = nc.dram_tensor("mT", [TC, P, TW], bf16, kind="ExternalInput").ap()
    out_dt = bf16 if OUT_BF16 else f32
    outT = nc.dram_tensor("outT", [D_OUT, NS], out_dt, kind="ExternalOutput").ap()

    with tile.TileContext(nc) as tc, ExitStack() as ctx:
        warm_pool = ctx.enter_context(tc.tile_pool(name="warm", bufs=1))
        aq_pool = ctx.enter_context(tc.tile_pool(name="aq", bufs=1))
        wq_pool = ctx.enter_context(tc.tile_pool(name="wq", bufs=1))
        bf_pool = ctx.enter_context(tc.tile_pool(name="bfp", bufs=1))
        bias_pool = ctx.enter_context(tc.tile_pool(name="bias", bufs=1))
        mask_pool = ctx.enter_context(tc.tile_pool(name="mask", bufs=4))
        # chunk-0 slab-loaded input tiles (stay resident; the w0 tile serves
        # as the g=0 weight tile for every chunk)
        xq0_pool = ctx.enter_context(tc.tile_pool(name="xq0", bufs=1))
        x0_pool = ctx.enter_context(tc.tile_pool(name="x0", bufs=1))
        w0_pool = ctx.enter_context(tc.tile_pool(name="w0", bufs=1))
        wt_pool = ctx.enter_context(tc.tile_pool(name="wt", bufs=WG - 1))
        xq_pool = ctx.enter_context(tc.tile_pool(name="xq", bufs=4))
        x_pool = ctx.enter_context(tc.tile_pool(name="x", bufs=4))
        u_pool = ctx.enter_context(tc.tile_pool(name="u", bufs=2))
        o_pool = ctx.enter_context(tc.tile_pool(name="o", bufs=4))
        pw_pool = ctx.enter_context(tc.tile_pool(name="pw", bufs=1, space="PSUM"))
        pu_pool = ctx.enter_context(tc.tile_pool(name="pu", bufs=2, space="PSUM"))
        po_pool = ctx.enter_context(tc.tile_pool(name="po", bufs=5, space="PSUM"))

        # Short PE warmup: covers the first small DMAs and starts the HAM
        # clock ramp while chunk-0 slices stream in.
        warm = warm_pool.tile([P, P], bf16)
        nc.gpsimd.memset(warm[:], 0.0)
        pw = pw_pool.tile([P, P], mybir.dt.float32)
        for _ in range(WARM):
            nc.tensor.matmul(pw[:], warm[:], warm[:], start=True, stop=True)

        def load_mask(t, gate=None):
            mk = mask_pool.tile([P, TW], bf16, tag="mk", name="mk")
            dma = nc.sync.dma_start(mk[:], mT[t])
            if gate is not None:
                add_dep_helper(dma.ins, gate.ins, sync=True, reason="pace")
            return mk

        # sync ring: small critical-path tensors, then chunk-0 slabs.
        aq = aq_pool.tile([P, KD, 2, LR], f8)
        nc.sync.dma_start(aq[:], aQ[:, :, :, :])
        xq0_t = xq0_pool.tile([P, KD, 2, TW], f8)
        for lo, hi in ((0, 3), (3, 6), (6, KD)):
            nc.sync.dma_start(xq0_t[:, lo:hi, :, :], xQ[0, :, lo:hi, :, :])
        mk0 = load_mask(0)
        bias_t = bias_pool.tile([P, OI], f32)
        nc.sync.dma_start(bias_t[:], bias[:, :])
        x0_t = x0_pool.tile([P, KB, TW], bf16)
        for lo in range(0, KB, 3):
            hi = min(lo + 3, KB)
            nc.sync.dma_start(x0_t[:, lo:hi, :], xT[0, :, lo:hi, :])

        # scalar ring: fp8 base weights, then g0 slabs (quad0 only needs
        # slots 2..11, so slot-2-first ordering feeds the kb-outer loop).
        wq = wq_pool.tile([P, KQMAX, 2, D_OUT], f8)
        nc.scalar.dma_start(wq[:], wQ[:, :, :, :])
        # group 0 serves only kq=3 rows (oi 0..3) -> slots 0,1 never read.
        w0_t = w0_pool.tile([P, KB, WGC], bf16)
        w0lo = 2 if (TRIM_WT and N3 >= 4) else 0
        for lo, hi in ((w0lo, 5), (5, 8), (8, KB)):
            nc.scalar.dma_start(w0_t[:, lo:hi, :], wT[0, :, lo:hi, :])
        bf_t = bf_pool.tile([P, D_OUT], bf16)
        bf_dma = nc.scalar.dma_start(bf_t[:], bF[:, :])
        # g1..g3 ride the idle GpSimd SWDGE queue: big transfers where the
        # ~2us software setup cost is amortized, keeping the two HWDGE rings
        # free for the latency-critical startup slabs. Their triggers (and
        # bF's) are gated behind the first down-projection matmul so the
        # chunk-0 critical slabs monopolize HBM bandwidth first.
        late_dmas = [bf_dma]
        wts = [w0_t]
        for g in range(1, WG):
            wt_g = wt_pool.tile([P, KB, WGC], bf16)
            # groups 1,2 serve only kq=3 rows -> slots 0,1 never read.
            lo = 2 if (TRIM_WT and N3 >= (g + 1) * 4) else 0
            late_dmas.append(
                nc.gpsimd.dma_start(wt_g[:, lo:, :], wT[g, :, lo:, :])
            )
            wts.append(wt_g)

        def load_chunk(t, gate=None, split=False):
            xq_c = xq_pool.tile([P, KD, 2, TW], f8)
            xb_c = x_pool.tile([P, KB, TW], bf16)
            dmas = []
            if split:
                for lo, hi in ((0, 3), (3, 6), (6, KD)):
                    dmas.append(
                        nc.sync.dma_start(xq_c[:, lo:hi, :, :], xQ[t, :, lo:hi, :, :])
                    )
                for lo in range(0, KB, 3):
                    hi = min(lo + 3, KB)
                    dmas.append(
                        nc.sync.dma_start(xb_c[:, lo:hi, :], xT[t, :, lo:hi, :])
                    )
            else:
                dmas.append(nc.sync.dma_start(xq_c[:], xQ[t]))
                dmas.append(nc.sync.dma_start(xb_c[:], xT[t]))
            if gate is not None:
                for d in dmas:
                    add_dep_helper(d.ins, gate.ins, sync=True, reason="pace")
            return (
                lambda kk, _x=xq_c: _x[:, kk, :, :],
                lambda kb, _x=xb_c: _x[:, kb, :],
            )

        def wslice(g, kb, loc):
            return wts[g][:, kb, loc : loc + P]

        def down(xq_slice, mk, after_first=None):
            """8 fp8 DoubleRow MMs + masked select; returns (um, first_mm)."""
            pu = pu_pool.tile([P, TW], mybir.dt.float32, tag="pu", name="pu")
            first = None
            for kk in range(KD):
                dmm = nc.tensor.matmul(
                    pu[:],
                    aq[:, kk, :, :],
                    xq_slice(kk),
                    start=(kk == 0),
                    stop=(kk == KD - 1),
                    perf_mode=DR,
                )
                if kk == 0:
                    first = dmm
                    if after_first is not None:
                        after_first()
            um = u_pool.tile([P, TW], bf16, tag="um", name="um")
            nc.vector.tensor_tensor(um[:], pu[:], mk[:], op=mybir.AluOpType.mult)
            return um, first

        def down_pair(xqa, xqb, mka, mkb):
            """Paired down-projection: each aq LDWEIGHTS serves 2 matmuls."""
            pu_a = pu_pool.tile([P, TW], mybir.dt.float32, tag="pu", name="pu")
            pu_b = pu_pool.tile([P, TW], mybir.dt.float32, tag="pu", name="pu")
            for kk in range(KD):
                st, sp = kk == 0, kk == KD - 1
                nc.tensor.matmul(
                    pu_a[:], aq[:, kk, :, :], xqa(kk), start=st, stop=sp,
                    perf_mode=DR,
                )
                nc.tensor.matmul(
                    pu_b[:], aq[:, kk, :, :], xqb(kk), start=st, stop=sp,
                    perf_mode=DR,
                )
            um_a = u_pool.tile([P, TW], bf16, tag="um", name="um")
            um_b = u_pool.tile([P, TW], bf16, tag="um", name="um")
            nc.vector.tensor_tensor(um_a[:], pu_a[:], mka[:], op=mybir.AluOpType.mult)
            nc.vector.tensor_tensor(um_b[:], pu_b[:], mkb[:], op=mybir.AluOpType.mult)
            return um_a, um_b

        def open_group(oi, xq_slice):
            # fp8 DoubleRow pairs cover contraction dims 0..256*kq-1.
            po = po_pool.tile([P, TW], mybir.dt.float32, tag="po", name="po")
            for kk in range(_kq_of(oi)):
                nc.tensor.matmul(
                    po[:],
                    wq[:, kk, :, oi * P : (oi + 1) * P],
                    xq_slice(kk),
                    start=(kk == 0),
                    stop=False,
                    perf_mode=DR,
                )
            return po

        def finish_group(t, oi, po, um, eng=None, split_out=False, veng=False):
            if not OUT_SPLIT:
                eng = None
                split_out = False
            up = nc.tensor.matmul(
                po[:],
                bf_t[:, oi * P : (oi + 1) * P],
                um[:],
                start=False,
                stop=True,
            )
            ot = o_pool.tile([P, TW], out_dt, tag="ot", name="ot")
            if veng and VEC_DRAIN:  # drain this member's PSUM on the DVE so
                # the two pair members' epilogues run on different engines
                nc.vector.tensor_scalar_add(ot[:], po[:], bias_t[:, oi : oi + 1])
            else:
                nc.scalar.add(ot[:], po[:], bias_t[:, oi : oi + 1])
            rows = slice(oi * P, (oi + 1) * P)
            if split_out:  # halve the final drain across both rings
                h = TW // 2
                c0 = t * TW
                nc.sync.dma_start(outT[rows, c0 : c0 + h], ot[:, 0:h])
                nc.scalar.dma_start(outT[rows, c0 + h : c0 + TW], ot[:, h:TW])
            else:
                (eng or nc.sync).dma_start(
                    outT[rows, t * TW : (t + 1) * TW], ot[:]
                )
            return up

        def solo_group(t, oi, xqs, xbs, um):
            # bf16-lead with spaced fp8 opens (same LDW pipelining as pairs)
            g = oi // WG
            loc = (oi % WG) * P
            kq = _kq_of(oi)
            kbs = list(range(_kb0_of(oi), KB))
            po = po_pool.tile([P, TW], mybir.dt.float32, tag="po", name="po")
            if SOLO_SPACED:
                nc.tensor.matmul(
                    po[:], wslice(g, kbs[0], loc), xbs(kbs[0]),
                    start=True, stop=False,
                )
                ki = 1
                for kk in range(kq):
                    nc.tensor.matmul(
                        po[:], wq[:, kk, :, oi * P : (oi + 1) * P], xqs(kk),
                        start=False, stop=False, perf_mode=DR,
                    )
                    if kk < kq - 1:
                        nc.tensor.matmul(
                            po[:], wslice(g, kbs[ki], loc), xbs(kbs[ki]),
                            start=False, stop=False,
                        )
                        ki += 1
                rest = kbs[ki:]
            else:
                for kk in range(kq):
                    nc.tensor.matmul(
                        po[:], wq[:, kk, :, oi * P : (oi + 1) * P], xqs(kk),
                        start=(kk == 0), stop=False, perf_mode=DR,
                    )
                rest = kbs
            for kb in rest:
                nc.tensor.matmul(
                    po[:], wslice(g, kb, loc), xbs(kb), start=False, stop=False
                )
            return finish_group(
                t, oi, po, um,
                eng=(nc.sync, nc.scalar)[oi % 2], veng=(oi % 2 == 1),
            )

        # ---- chunk 0 solo: kb-outer quad 0, then quads 1..3 ----
        xq0_slice = lambda kk: xq0_t[:, kk, :, :]  # noqa: E731
        xb0_slice = lambda kb: x0_t[:, kb, :]  # noqa: E731

        quad0_po = []

        def _open_quad0():
            # Interleave quad-0's fp8 group openers into the DMA-paced
            # down-projection window.
            for oi in range(4):
                quad0_po.append(open_group(oi, xq0_slice))

        um0, down0_first = down(xq0_slice, mk0, after_first=_open_quad0)
        if GATE_LATE:
            for dma in late_dmas:
                add_dep_helper(dma.ins, down0_first.ins, sync=True, reason="pace")

        # Filler matmuls keep the PE busy (and HAM warm) while the chunk-0
        # bf16 slabs stream in; they run only if quad-0 data hasn't landed.
        for _ in range(FILL):
            nc.tensor.matmul(pw[:], warm[:], warm[:], start=True, stop=True)

        # quad 0 @ t0: kb-outer so the PE consumes each k-slice on arrival.
        # (quad-0 groups all have kq=3 -> bf16 slots 2..11.)
        for kb in range(_kb0_of(0), KB):
            for oi in range(4):
                nc.tensor.matmul(
                    quad0_po[oi],
                    wslice(0, kb, oi * P),
                    xb0_slice(kb),
                    start=False,
                    stop=False,
                )
        quad0_last_up = None
        for oi in range(4):
            quad0_last_up = finish_group(
                0, oi, quad0_po[oi], um0, eng=(nc.sync, nc.scalar)[oi % 2]
            )
        for oi in range(4, OI):
            solo_group(0, oi, xq0_slice, xb0_slice, um0)

        # ---- chunk 1 solo (loads gated until quad 0 is done so the W
        # groups get the early HBM bandwidth) ----
        c1_gate = quad0_last_up if GATE_C1 else None
        xq1_slice, xb1_slice = load_chunk(1, gate=c1_gate, split=True)
        mk1 = load_mask(1, gate=c1_gate)
        um1, down1_first = down(xq1_slice, mk1)
        for oi in range(OI):
            solo_group(1, oi, xq1_slice, xb1_slice, um1)

        # ---- steady-state pairs (2,3), (4,5), (6,7) ----
        for tp in range(2, TC, 2):
            gate = down1_first if tp == 2 else None
            xqa, xba = load_chunk(tp, gate=gate)
            xqb, xbb = load_chunk(tp + 1, gate=gate)
            mka = load_mask(tp, gate=gate)
            mkb = load_mask(tp + 1, gate=gate)
            if PAIR_ILV:
                um_a, um_b = down_pair(xqa, xqb, mka, mkb)
            else:
                um_a, _ = down(xqa, mka)
                um_b, _ = down(xqb, mkb)
            for oi in range(OI):
                g = oi // WG
                loc = (oi % WG) * P
                kq = _kq_of(oi)
                kbs = list(range(_kb0_of(oi), KB))
                po_a = po_pool.tile([P, TW], mybir.dt.float32, tag="po", name="po")
                po_b = po_pool.tile([P, TW], mybir.dt.float32, tag="po", name="po")

                def bf_pair(kb, st=False):
                    ws = wslice(g, kb, loc)
                    nc.tensor.matmul(po_a[:], ws, xba(kb), start=st, stop=False)
                    nc.tensor.matmul(po_b[:], ws, xbb(kb), start=st, stop=False)

                def dr_pair(kk):
                    ws = wq[:, kk, :, oi * P : (oi + 1) * P]
                    nc.tensor.matmul(
                        po_a[:], ws, xqa(kk), start=False, stop=False,
                        perf_mode=DR,
                    )
                    nc.tensor.matmul(
                        po_b[:], ws, xqb(kk), start=False, stop=False,
                        perf_mode=DR,
                    )

                # bf16 leads (start=True); each DR open then follows a full
                # bf16-pair window so its 256-col LDWEIGHTS never stalls.
                ki = 0
                bf_pair(kbs[ki], st=True)
                ki += 1
                for kk in range(kq):
                    dr_pair(kk)
                    if kk < kq - 1:
                        bf_pair(kbs[ki])
                        ki += 1
                for kb in kbs[ki:]:
                    bf_pair(kb)
                ra, rb = (nc.sync, nc.scalar) if oi % 2 == 0 else (nc.scalar, nc.sync)
                last = tp + 1 == TC and oi >= OI - 6
                finish_group(tp, oi, po_a, um_a, eng=ra)
                finish_group(
                    tp + 1, oi, po_b, um_b, eng=rb, split_out=last, veng=True
                )

    nc.compile()
    return nc


def _get_nc():
    if "nc" not in _CACHE:
        _CACHE["nc"] = _build()
    return _CACHE["nc"]


def _install_trace_shim():
    """This image's antenv lacks axon_hooks; register the NTFF profile hook
    ourselves so run_bass_kernel_spmd(trace=True) can capture exec_time_ns."""
    import sys
    import types

    if "antenv.axon_hooks" in sys.modules:
        return
    import antenv

    mod = types.ModuleType("antenv.axon_hooks")
    state = {"hook": None}
    mod.set_axon_ntff_profile_hook = lambda h: state.__setitem__("hook", h)
    mod.get_axon_ntff_profile_hook = lambda: state["hook"]
    sys.modules["antenv.axon_hooks"] = mod
    antenv.axon_hooks = mod

    from trn_agent_boot.trn_boot import _ntff_profile_via_ctypes

    mod.set_axon_ntff_profile_hook(
        _ntff_profile_via_ctypes("/opt/axon/libaxon_pjrt.so")
    )

    # No S3 in this container; keep artifacts local.
    import concourse.bass_utils as bu

    bu.upload_artifacts = lambda tmpdir: f"local://{tmpdir}"


def _q8(a):
    return np.clip(a, -240.0, 240.0).astype(_F8)


def kernel(x, W, b, A_all, B_all, lora_idx, _trace=False):
    global LAST_EXEC_TIME_NS
    from concourse.bass_utils import run_bass_kernel_spmd

    if _trace:
        try:
            _install_trace_shim()
        except Exception as e:  # degrade to untraced run
            print(f"trace shim failed ({e!r}); running untraced")
            _trace = False

    x = np.asarray(x, dtype=np.float32)
    W = np.asarray(W, dtype=np.float32)
    b = np.asarray(b, dtype=np.float32)
    A_all = np.asarray(A_all, dtype=np.float32)
    B_all = np.asarray(B_all, dtype=np.float32)
    lora_idx = np.asarray(lora_idx, dtype=np.int32)

    # Host-side weight reformat (replicated across cores), partition-major.
    w4 = W.reshape(WG, WGC, KT, P).transpose(0, 3, 2, 1)  # [g,p,k,o]
    wT_np = np.ascontiguousarray((w4[:, :, 4:, :] * GS).astype(_BF16))
    wQ_np = np.ascontiguousarray(
        _q8(W[:, : 2 * KQMAX * P].reshape(D_OUT, KQMAX, 2, P) * SQ).transpose(
            3, 1, 2, 0
        )
    )
    A_flat = A_all.reshape(LR, D_IN)
    aQ_np = np.ascontiguousarray(
        _q8(A_flat.reshape(LR, KD, 2, P) * SQ).transpose(3, 1, 2, 0)
    )
    bF_np = np.ascontiguousarray(B_all.transpose(0, 2, 1)).reshape(LR, D_OUT).astype(
        _BF16
    )
    bias_np = np.ascontiguousarray((b * GS).reshape(OI, P).T).astype(np.float32)

    adapters = (np.arange(LR, dtype=np.int32) // R)[:, None]  # [LR, 1]

    in_maps = []
    for i in range(M_CORES):
        s = slice(i * NS, (i + 1) * NS)
        xr = x[s].reshape(TC, TW, KT, P)
        xT_i = np.ascontiguousarray(
            xr[:, :, 4:, :].astype(_BF16).transpose(0, 3, 2, 1)
        )
        xQ_i = np.ascontiguousarray(
            _q8(xr.reshape(TC, TW, KD, 2, P) * SX).transpose(0, 4, 2, 3, 1)
        )
        idx = lora_idx[s]
        mfull = (adapters == idx[None, :]).astype(np.float32) * SCALE  # [LR, NS]
        mT_i = np.ascontiguousarray(
            mfull.astype(_BF16).reshape(LR, TC, TW).transpose(1, 0, 2)
        )
        in_maps.append(
            {
                "xT": xT_i,
                "xQ": xQ_i,
                "wT": wT_np,
                "wQ": wQ_np,
                "aQ": aQ_np,
                "bF": bF_np,
                "bias": bias_np,
                "mT": mT_i,
            }
        )

    nc = _get_nc()
    res = run_bass_kernel_spmd(
        nc, in_maps, core_ids=list(range(M_CORES)), trace=_trace
    )
    LAST_EXEC_TIME_NS = res.exec_time_ns

    out = np.empty((N, D_OUT), dtype=np.float32)
    inv = np.float32(1.0 / GS)
    for i in range(M_CORES):
        r = res.results[i]["outT"]
        out[i * NS : (i + 1) * NS] = r.T.astype(np.float32) * inv
    return out


# revision 28
# speedup vs baseline: 1.0084x; 1.0084x over previous
"""Fused multi-LoRA linear layer on 8 TRN2 NeuronCores.

out = x @ W.T + b + scale * mask(x @ A_all^T) @ B_flat

Sharding: data-parallel over the token dim N (32768 -> 8 x 4096).
Weights (W, A_all, B_all, b) are replicated; each core computes its token
shard fully, so no collectives are needed.

v5: wider fp8 coverage tuned to both the error budget and the chip's
power-state ceiling, plus LDWEIGHTS pair-sharing and ring balancing.
- 13 of 16 output row-groups run 3 fp8e4 DoubleRow k-pairs (contraction
  dims 0..767); the rest use 2 pairs. Host-side bit-exact simulation puts
  rel err at 1.934e-2 (sim matches HW to ~6 digits) under the 2e-2 gate.
- IMPORTANT: pushing to 14 KQ=3 groups (432 DR matmuls/core vs 424) trips
  a power-profile downclock -- the PE drops from 2.4 to ~2.0 GHz for the
  whole run (all matmuls 216 -> 259 ns) and the kernel LOSES ~80us. The
  DR-matmul density ceiling binds before the accuracy budget does.
- Down-projection and fp8 open matmuls of a chunk pair interleave so each
  256-col DR LDWEIGHTS serves two matmuls (halves DR weight-load stalls).
- Output is written bf16 (halves output HBM traffic; +3e-6 error var) and
  output DMAs alternate between the sync and scalar HWDGE rings; the last
  chunk's tail DMAs split across both rings.
- Startup: warm-tile memset on GpSimd, warmup+filler matmuls bridge the
  chunk-0 DMA ramp so HAM never re-throttles; chunk-1 loads are split so
  its down-projection starts on first-piece arrival.
"""

import numpy as np
import ml_dtypes

# Problem constants (hardcoded per harness contract).
N, D_IN, D_OUT, L, R = 32768, 2048, 2048, 8, 16
SCALE = 32.0 / 16.0
M_CORES = 8
NS = N // M_CORES  # 4096 tokens per core
P = 128
KT = D_IN // P  # 16 k-tiles
KQMAX = 3  # max fp8 DoubleRow k-PAIRS in the base matmul
N3 = 13  # output row-groups (of OI) using KQMAX pairs; the rest use 2
KB = KT - 4  # bf16 k-tile slots (k-tiles 4..15); KQ=3 groups skip the first 2
KD = KT // 2  # fp8 k-pairs in the down-projection (all 8)
OI = D_OUT // P  # 16 output row-chunks of 128
TW = 512  # token tile width (moving free dim)
TC = NS // TW  # 8 token chunks per core
LR = L * R  # 128
WG = 4  # W column groups
WGC = D_OUT // WG  # 512 columns per group
WARM = 112  # PE warmup matmuls before the first real matmul
FILL = 96  # startup filler matmuls (bridge chunk-0 DMA window)
PAIR_ILV = True  # interleave chunk-pair DR matmuls (share LDWEIGHTS)
GATE_C1 = True  # gate chunk-1 loads behind quad-0 completion
GATE_LATE = True  # gate bF/g1..3 slabs behind the first down matmul
OUT_SPLIT = True  # alternate output DMAs between sync/scalar rings
OUT_BF16 = True  # write outT in bf16 (halves output HBM traffic)
TRIM_WT = True  # skip DMA of never-read bf16 weight slots
SWDGE_Q = 1  # GpSimd software-DGE queues
VEC_DRAIN = False  # vector-engine PSUM drain: measured no faster
SOLO_SPACED = False  # spaced solo-chunk opens: measured ~1us slower

SX = 32.0  # fp8 scale for x
SQ = 2048.0  # fp8 scale for W and A
GS = SX * SQ  # 2^16: global scale of all device-side math

_BF16 = ml_dtypes.bfloat16
_F8 = ml_dtypes.float8_e4m3

_CACHE = {}

LAST_EXEC_TIME_NS = None


def _kq_of(oi):
    return KQMAX if oi < N3 else 2


def _kb0_of(oi):
    # first usable bf16 k-tile slot (slot kb holds k-tile kb+4)
    return 2 if oi < N3 else 0


def _build():
    import concourse.bass as bass  # noqa: F401
    import concourse.tile as tile
    from concourse import bacc, mybir
    from concourse.tile_rust import add_dep_helper
    from contextlib import ExitStack

    bf16 = mybir.dt.bfloat16
    f8 = mybir.dt.float8e4
    f32 = mybir.dt.float32
    DR = mybir.MatmulPerfMode.DoubleRow

    nc = bacc.Bacc(
        "TRN2",
        target_bir_lowering=False,
        debug=False,
        num_devices=M_CORES,
        num_swdge_queues=SWDGE_Q,
    )

    # Host-prepared, partition-major layouts (see kernel()):
    #   xT [TC, P, KB, TW] bf16 : xT[t,p,kb,j] = x[t*TW+j, (kb+4)*P+p]
    #   xQ [TC, P, KD, 2, TW] f8: xQ[t,p,kk,u,j] = q8(x[t*TW+j, (2kk+u)*P+p]*SX)
    #   wT [WG, P, KB, WGC] bf16: wT[g,p,kb,o] = W[g*WGC+o, (kb+4)*P+p]*GS
    #   wQ [P, KQMAX, 2, D_OUT] f8: wQ[p,kk,u,o] = q8(W[o, (2kk+u)*P+p]*SQ)
    #   aQ [P, KD, 2, LR] f8    : aQ[p,kk,u,c] = q8(A_flat[c, (2kk+u)*P+p]*SQ)
    #   bF [P, D_OUT] bf16      : bF[c,o] = B_all[c//R, o, c%R]
    #   bias [P, OI] f32        : bias[p,oi] = b[oi*P+p]*GS
    #   mT [TC, P, TW] bf16     : one-hot adapter mask * SCALE
    xT = nc.dram_tensor("xT", [TC, P, KB, TW], bf16, kind="ExternalInput").ap()
    xQ = nc.dram_tensor("xQ", [TC, P, KD, 2, TW], f8, kind="ExternalInput").ap()
    wT = nc.dram_tensor("wT", [WG, P, KB, WGC], bf16, kind="ExternalInput").ap()
    wQ = nc.dram_tensor("wQ", [P, KQMAX, 2, D_OUT], f8, kind="ExternalInput").ap()
    aQ = nc.dram_tensor("aQ", [P, KD, 2, LR], f8, kind="ExternalInput").ap()
    bF = nc.dram_tensor("bF", [P, D_OUT], bf16, kind="ExternalInput").ap()
    bias = nc.dram_tensor("bias", [P, OI], f32, kind="ExternalInput").ap()
    mT = nc.dram_tensor("mT", [TC, P, TW], bf16, kind="ExternalInput").ap()
    out_dt = bf16 if OUT_BF16 else f32
    outT = nc.dram_tensor("outT", [D_OUT, NS], out_dt, kind="ExternalOutput").ap()

    with tile.TileContext(nc) as tc, ExitStack() as ctx:
        warm_pool = ctx.enter_context(tc.tile_pool(name="warm", bufs=1))
        aq_pool = ctx.enter_context(tc.tile_pool(name="aq", bufs=1))
        wq_pool = ctx.enter_context(tc.tile_pool(name="wq", bufs=1))
        bf_pool = ctx.enter_context(tc.tile_pool(name="bfp", bufs=1))
        bias_pool = ctx.enter_context(tc.tile_pool(name="bias", bufs=1))
        mask_pool = ctx.enter_context(tc.tile_pool(name="mask", bufs=4))
        # chunk-0 slab-loaded input tiles (stay resident; the w0 tile serves
        # as the g=0 weight tile for every chunk)
        xq0_pool = ctx.enter_context(tc.tile_pool(name="xq0", bufs=1))
        x0_pool = ctx.enter_context(tc.tile_pool(name="x0", bufs=1))
        w0_pool = ctx.enter_context(tc.tile_pool(name="w0", bufs=1))
        wt_pool = ctx.enter_context(tc.tile_pool(name="wt", bufs=WG - 1))
        xq_pool = ctx.enter_context(tc.tile_pool(name="xq", bufs=4))
        x_pool = ctx.enter_context(tc.tile_pool(name="x", bufs=4))
        u_pool = ctx.enter_context(tc.tile_pool(name="u", bufs=2))
        o_pool = ctx.enter_context(tc.tile_pool(name="o", bufs=4))
        pw_pool = ctx.enter_context(tc.tile_pool(name="pw", bufs=1, space="PSUM"))
        pu_pool = ctx.enter_context(tc.tile_pool(name="pu", bufs=2, space="PSUM"))
        po_pool = ctx.enter_context(tc.tile_pool(name="po", bufs=5, space="PSUM"))

        # Short PE warmup: covers the first small DMAs and starts the HAM
        # clock ramp while chunk-0 slices stream in.
        warm = warm_pool.tile([P, P], bf16)
        nc.gpsimd.memset(warm[:], 0.0)
        pw = pw_pool.tile([P, P], mybir.dt.float32)
        for _ in range(WARM):
            nc.tensor.matmul(pw[:], warm[:], warm[:], start=True, stop=True)

        def load_mask(t, gate=None):
            mk = mask_pool.tile([P, TW], bf16, tag="mk", name="mk")
            dma = nc.sync.dma_start(mk[:], mT[t])
            if gate is not None:
                add_dep_helper(dma.ins, gate.ins, sync=True, reason="pace")
            return mk

        # sync ring: small critical-path tensors, then chunk-0 slabs.
        aq = aq_pool.tile([P, KD, 2, LR], f8)
        nc.sync.dma_start(aq[:], aQ[:, :, :, :])
        xq0_t = xq0_pool.tile([P, KD, 2, TW], f8)
        for lo, hi in ((0, 3), (3, 6), (6, KD)):
            nc.sync.dma_start(xq0_t[:, lo:hi, :, :], xQ[0, :, lo:hi, :, :])
        mk0 = load_mask(0)
        bias_t = bias_pool.tile([P, OI], f32)
        nc.sync.dma_start(bias_t[:], bias[:, :])
        x0_t = x0_pool.tile([P, KB, TW], bf16)
        for lo in range(0, KB, 3):
            hi = min(lo + 3, KB)
            nc.sync.dma_start(x0_t[:, lo:hi, :], xT[0, :, lo:hi, :])

        # scalar ring: fp8 base weights, then g0 slabs (quad0 only needs
        # slots 2..11, so slot-2-first ordering feeds the kb-outer loop).
        wq = wq_pool.tile([P, KQMAX, 2, D_OUT], f8)
        nc.scalar.dma_start(wq[:], wQ[:, :, :, :])
        # group 0 serves only kq=3 rows (oi 0..3) -> slots 0,1 never read.
        w0_t = w0_pool.tile([P, KB, WGC], bf16)
        w0lo = 2 if (TRIM_WT and N3 >= 4) else 0
        for lo, hi in ((w0lo, 5), (5, 8), (8, KB)):
            nc.scalar.dma_start(w0_t[:, lo:hi, :], wT[0, :, lo:hi, :])
        bf_t = bf_pool.tile([P, D_OUT], bf16)
        bf_dma = nc.scalar.dma_start(bf_t[:], bF[:, :])
        # g1..g3 ride the idle GpSimd SWDGE queue: big transfers where the
        # ~2us software setup cost is amortized, keeping the two HWDGE rings
        # free for the latency-critical startup slabs. Their triggers (and
        # bF's) are gated behind the first down-projection matmul so the
        # chunk-0 critical slabs monopolize HBM bandwidth first.
        late_dmas = [bf_dma]
        wts = [w0_t]
        for g in range(1, WG):
            wt_g = wt_pool.tile([P, KB, WGC], bf16)
            # groups 1,2 serve only kq=3 rows -> slots 0,1 never read.
            lo = 2 if (TRIM_WT and N3 >= (g + 1) * 4) else 0
            late_dmas.append(
                nc.gpsimd.dma_start(wt_g[:, lo:, :], wT[g, :, lo:, :])
            )
            wts.append(wt_g)

        def load_chunk(t, gate=None, split=False):
            xq_c = xq_pool.tile([P, KD, 2, TW], f8)
            xb_c = x_pool.tile([P, KB, TW], bf16)
            dmas = []
            if split:
                for lo, hi in ((0, 3), (3, 6), (6, KD)):
                    dmas.append(
                        nc.sync.dma_start(xq_c[:, lo:hi, :, :], xQ[t, :, lo:hi, :, :])
                    )
                for lo in range(0, KB, 3):
                    hi = min(lo + 3, KB)
                    dmas.append(
                        nc.sync.dma_start(xb_c[:, lo:hi, :], xT[t, :, lo:hi, :])
                    )
            else:
                dmas.append(nc.sync.dma_start(xq_c[:], xQ[t]))
                dmas.append(nc.sync.dma_start(xb_c[:], xT[t]))
            if gate is not None:
                for d in dmas:
                    add_dep_helper(d.ins, gate.ins, sync=True, reason="pace")
            return (
                lambda kk, _x=xq_c: _x[:, kk, :, :],
                lambda kb, _x=xb_c: _x[:, kb, :],
            )

        def wslice(g, kb, loc):
            return wts[g][:, kb, loc : loc + P]

        def down(xq_slice, mk, after_first=None):
            """8 fp8 DoubleRow MMs + masked select; returns (um, first_mm)."""
            pu = pu_pool.tile([P, TW], mybir.dt.float32, tag="pu", name="pu")
            first = None
            for kk in range(KD):
                dmm = nc.tensor.matmul(
                    pu[:],
                    aq[:, kk, :, :],
                    xq_slice(kk),
                    start=(kk == 0),
                    stop=(kk == KD - 1),
                    perf_mode=DR,
                )
                if kk == 0:
                    first = dmm
                    if after_first is not None:
                        after_first()
            um = u_pool.tile([P, TW], bf16, tag="um", name="um")
            nc.vector.tensor_tensor(um[:], pu[:], mk[:], op=mybir.AluOpType.mult)
            return um, first

        def down_pair(xqa, xqb, mka, mkb):
            """Paired down-projection: each aq LDWEIGHTS serves 2 matmuls."""
            pu_a = pu_pool.tile([P, TW], mybir.dt.float32, tag="pu", name="pu")
            pu_b = pu_pool.tile([P, TW], mybir.dt.float32, tag="pu", name="pu")
            for kk in range(KD):
                st, sp = kk == 0, kk == KD - 1
                nc.tensor.matmul(
                    pu_a[:], aq[:, kk, :, :], xqa(kk), start=st, stop=sp,
                    perf_mode=DR,
                )
                nc.tensor.matmul(
                    pu_b[:], aq[:, kk, :, :], xqb(kk), start=st, stop=sp,
                    perf_mode=DR,
                )
            um_a = u_pool.tile([P, TW], bf16, tag="um", name="um")
            um_b = u_pool.tile([P, TW], bf16, tag="um", name="um")
            nc.vector.tensor_tensor(um_a[:], pu_a[:], mka[:], op=mybir.AluOpType.mult)
            nc.vector.tensor_tensor(um_b[:], pu_b[:], mkb[:], op=mybir.AluOpType.mult)
            return um_a, um_b

        def open_group(oi, xq_slice):
            # fp8 DoubleRow pairs cover contraction dims 0..256*kq-1.
            po = po_pool.tile([P, TW], mybir.dt.float32, tag="po", name="po")
            for kk in range(_kq_of(oi)):
                nc.tensor.matmul(
                    po[:],
                    wq[:, kk, :, oi * P : (oi + 1) * P],
                    xq_slice(kk),
                    start=(kk == 0),
                    stop=False,
                    perf_mode=DR,
                )
            return po

        def finish_group(t, oi, po, um, eng=None, split_out=False, veng=False):
            if not OUT_SPLIT:
                eng = None
                split_out = False
            up = nc.tensor.matmul(
                po[:],
                bf_t[:, oi * P : (oi + 1) * P],
                um[:],
                start=False,
                stop=True,
            )
            ot = o_pool.tile([P, TW], out_dt, tag="ot", name="ot")
            if veng and VEC_DRAIN:  # drain this member's PSUM on the DVE so
                # the two pair members' epilogues run on different engines
                nc.vector.tensor_scalar_add(ot[:], po[:], bias_t[:, oi : oi + 1])
            else:
                nc.scalar.add(ot[:], po[:], bias_t[:, oi : oi + 1])
            rows = slice(oi * P, (oi + 1) * P)
            if split_out:  # halve the final drain across both rings
                h = TW // 2
                c0 = t * TW
                nc.sync.dma_start(outT[rows, c0 : c0 + h], ot[:, 0:h])
                nc.scalar.dma_start(outT[rows, c0 + h : c0 + TW], ot[:, h:TW])
            else:
                (eng or nc.sync).dma_start(
                    outT[rows, t * TW : (t + 1) * TW], ot[:]
                )
            return up

        def solo_group(t, oi, xqs, xbs, um):
            # bf16-lead with spaced fp8 opens (same LDW pipelining as pairs)
            g = oi // WG
            loc = (oi % WG) * P
            kq = _kq_of(oi)
            kbs = list(range(_kb0_of(oi), KB))
            po = po_pool.tile([P, TW], mybir.dt.float32, tag="po", name="po")
            if SOLO_SPACED:
                nc.tensor.matmul(
                    po[:], wslice(g, kbs[0], loc), xbs(kbs[0]),
                    start=True, stop=False,
                )
                ki = 1
                for kk in range(kq):
                    nc.tensor.matmul(
                        po[:], wq[:, kk, :, oi * P : (oi + 1) * P], xqs(kk),
                        start=False, stop=False, perf_mode=DR,
                    )
                    if kk < kq - 1:
                        nc.tensor.matmul(
                            po[:], wslice(g, kbs[ki], loc), xbs(kbs[ki]),
                            start=False, stop=False,
                        )
                        ki += 1
                rest = kbs[ki:]
            else:
                for kk in range(kq):
                    nc.tensor.matmul(
                        po[:], wq[:, kk, :, oi * P : (oi + 1) * P], xqs(kk),
                        start=(kk == 0), stop=False, perf_mode=DR,
                    )
                rest = kbs
            for kb in rest:
                nc.tensor.matmul(
                    po[:], wslice(g, kb, loc), xbs(kb), start=False, stop=False
                )
            return finish_group(
                t, oi, po, um, eng=(nc.sync, nc.scalar)[oi % 2]
            )

        # ---- chunk 0 solo: kb-outer quad 0, then quads 1..3 ----
        xq0_slice = lambda kk: xq0_t[:, kk, :, :]  # noqa: E731
        xb0_slice = lambda kb: x0_t[:, kb, :]  # noqa: E731

        quad0_po = []

        def _open_quad0():
            # Interleave quad-0's fp8 group openers into the DMA-paced
            # down-projection window.
            for oi in range(4):
                quad0_po.append(open_group(oi, xq0_slice))

        um0, down0_first = down(xq0_slice, mk0, after_first=_open_quad0)
        if GATE_LATE:
            for dma in late_dmas:
                add_dep_helper(dma.ins, down0_first.ins, sync=True, reason="pace")

        # Filler matmuls keep the PE busy (and HAM warm) while the chunk-0
        # bf16 slabs stream in; they run only if quad-0 data hasn't landed.
        for _ in range(FILL):
            nc.tensor.matmul(pw[:], warm[:], warm[:], start=True, stop=True)

        # quad 0 @ t0: kb-outer so the PE consumes each k-slice on arrival.
        # (quad-0 groups all have kq=3 -> bf16 slots 2..11.)
        for kb in range(_kb0_of(0), KB):
            for oi in range(4):
                nc.tensor.matmul(
                    quad0_po[oi],
                    wslice(0, kb, oi * P),
                    xb0_slice(kb),
                    start=False,
                    stop=False,
                )
        quad0_last_up = None
        for oi in range(4):
            quad0_last_up = finish_group(
                0, oi, quad0_po[oi], um0, eng=(nc.sync, nc.scalar)[oi % 2]
            )
        for oi in range(4, OI):
            solo_group(0, oi, xq0_slice, xb0_slice, um0)

        # ---- chunk 1 solo (loads gated until quad 0 is done so the W
        # groups get the early HBM bandwidth) ----
        c1_gate = quad0_last_up if GATE_C1 else None
        xq1_slice, xb1_slice = load_chunk(1, gate=c1_gate, split=True)
        mk1 = load_mask(1, gate=c1_gate)
        um1, down1_first = down(xq1_slice, mk1)
        for oi in range(OI):
            solo_group(1, oi, xq1_slice, xb1_slice, um1)

        # ---- steady-state pairs (2,3), (4,5), (6,7) ----
        for tp in range(2, TC, 2):
            gate = down1_first if tp == 2 else None
            xqa, xba = load_chunk(tp, gate=gate)
            xqb, xbb = load_chunk(tp + 1, gate=gate)
            mka = load_mask(tp, gate=gate)
            mkb = load_mask(tp + 1, gate=gate)
            if PAIR_ILV:
                um_a, um_b = down_pair(xqa, xqb, mka, mkb)
            else:
                um_a, _ = down(xqa, mka)
                um_b, _ = down(xqb, mkb)
            for oi in range(OI):
                g = oi // WG
                loc = (oi % WG) * P
                kq = _kq_of(oi)
                kbs = list(range(_kb0_of(oi), KB))
                po_a = po_pool.tile([P, TW], mybir.dt.float32, tag="po", name="po")
                po_b = po_pool.tile([P, TW], mybir.dt.float32, tag="po", name="po")

                def bf_pair(kb, st=False):
                    ws = wslice(g, kb, loc)
                    nc.tensor.matmul(po_a[:], ws, xba(kb), start=st, stop=False)
                    nc.tensor.matmul(po_b[:], ws, xbb(kb), start=st, stop=False)

                def dr_pair(kk):
                    ws = wq[:, kk, :, oi * P : (oi + 1) * P]
                    nc.tensor.matmul(
                        po_a[:], ws, xqa(kk), start=False, stop=False,
                        perf_mode=DR,
                    )
                    nc.tensor.matmul(
                        po_b[:], ws, xqb(kk), start=False, stop=False,
                        perf_mode=DR,
                    )

                # bf16 leads (start=True); each DR open then follows a full
                # bf16-pair window so its 256-col LDWEIGHTS never stalls.
                ki = 0
                bf_pair(kbs[ki], st=True)
                ki += 1
                for kk in range(kq):
                    dr_pair(kk)
                    if kk < kq - 1:
                        bf_pair(kbs[ki])
                        ki += 1
                for kb in kbs[ki:]:
                    bf_pair(kb)
                ra, rb = (nc.sync, nc.scalar) if oi % 2 == 0 else (nc.scalar, nc.sync)
                last = tp + 1 == TC and oi >= OI - 6
                finish_group(tp, oi, po_a, um_a, eng=ra)
                finish_group(
                    tp + 1, oi, po_b, um_b, eng=rb, split_out=last, veng=last
                )

    nc.compile()
    return nc


def _get_nc():
    if "nc" not in _CACHE:
        _CACHE["nc"] = _build()
    return _CACHE["nc"]


def _install_trace_shim():
    """This image's antenv lacks axon_hooks; register the NTFF profile hook
    ourselves so run_bass_kernel_spmd(trace=True) can capture exec_time_ns."""
    import sys
    import types

    if "antenv.axon_hooks" in sys.modules:
        return
    import antenv

    mod = types.ModuleType("antenv.axon_hooks")
    state = {"hook": None}
    mod.set_axon_ntff_profile_hook = lambda h: state.__setitem__("hook", h)
    mod.get_axon_ntff_profile_hook = lambda: state["hook"]
    sys.modules["antenv.axon_hooks"] = mod
    antenv.axon_hooks = mod

    from trn_agent_boot.trn_boot import _ntff_profile_via_ctypes

    mod.set_axon_ntff_profile_hook(
        _ntff_profile_via_ctypes("/opt/axon/libaxon_pjrt.so")
    )

    # No S3 in this container; keep artifacts local.
    import concourse.bass_utils as bu

    bu.upload_artifacts = lambda tmpdir: f"local://{tmpdir}"


def _q8(a):
    return np.clip(a, -240.0, 240.0).astype(_F8)


def kernel(x, W, b, A_all, B_all, lora_idx, _trace=False):
    global LAST_EXEC_TIME_NS
    from concourse.bass_utils import run_bass_kernel_spmd

    if _trace:
        try:
            _install_trace_shim()
        except Exception as e:  # degrade to untraced run
            print(f"trace shim failed ({e!r}); running untraced")
            _trace = False

    x = np.asarray(x, dtype=np.float32)
    W = np.asarray(W, dtype=np.float32)
    b = np.asarray(b, dtype=np.float32)
    A_all = np.asarray(A_all, dtype=np.float32)
    B_all = np.asarray(B_all, dtype=np.float32)
    lora_idx = np.asarray(lora_idx, dtype=np.int32)

    # Host-side weight reformat (replicated across cores), partition-major.
    w4 = W.reshape(WG, WGC, KT, P).transpose(0, 3, 2, 1)  # [g,p,k,o]
    wT_np = np.ascontiguousarray((w4[:, :, 4:, :] * GS).astype(_BF16))
    wQ_np = np.ascontiguousarray(
        _q8(W[:, : 2 * KQMAX * P].reshape(D_OUT, KQMAX, 2, P) * SQ).transpose(
            3, 1, 2, 0
        )
    )
    A_flat = A_all.reshape(LR, D_IN)
    aQ_np = np.ascontiguousarray(
        _q8(A_flat.reshape(LR, KD, 2, P) * SQ).transpose(3, 1, 2, 0)
    )
    bF_np = np.ascontiguousarray(B_all.transpose(0, 2, 1)).reshape(LR, D_OUT).astype(
        _BF16
    )
    bias_np = np.ascontiguousarray((b * GS).reshape(OI, P).T).astype(np.float32)

    adapters = (np.arange(LR, dtype=np.int32) // R)[:, None]  # [LR, 1]

    in_maps = []
    for i in range(M_CORES):
        s = slice(i * NS, (i + 1) * NS)
        xr = x[s].reshape(TC, TW, KT, P)
        xT_i = np.ascontiguousarray(
            xr[:, :, 4:, :].astype(_BF16).transpose(0, 3, 2, 1)
        )
        xQ_i = np.ascontiguousarray(
            _q8(xr.reshape(TC, TW, KD, 2, P) * SX).transpose(0, 4, 2, 3, 1)
        )
        idx = lora_idx[s]
        mfull = (adapters == idx[None, :]).astype(np.float32) * SCALE  # [LR, NS]
        mT_i = np.ascontiguousarray(
            mfull.astype(_BF16).reshape(LR, TC, TW).transpose(1, 0, 2)
        )
        in_maps.append(
            {
                "xT": xT_i,
                "xQ": xQ_i,
                "wT": wT_np,
                "wQ": wQ_np,
                "aQ": aQ_np,
                "bF": bF_np,
                "bias": bias_np,
                "mT": mT_i,
            }
        )

    nc = _get_nc()
    res = run_bass_kernel_spmd(
        nc, in_maps, core_ids=list(range(M_CORES)), trace=_trace
    )
    LAST_EXEC_TIME_NS = res.exec_time_ns

    out = np.empty((N, D_OUT), dtype=np.float32)
    inv = np.float32(1.0 / GS)
    for i in range(M_CORES):
        r = res.results[i]["outT"]
        out[i * NS : (i + 1) * NS] = r.T.astype(np.float32) * inv
    return out


# revision 30
# speedup vs baseline: 1.0086x; 1.0001x over previous
"""Fused multi-LoRA linear layer on 8 TRN2 NeuronCores.

out = x @ W.T + b + scale * mask(x @ A_all^T) @ B_flat

Sharding: data-parallel over the token dim N (32768 -> 8 x 4096).
Weights (W, A_all, B_all, b) are replicated; each core computes its token
shard fully, so no collectives are needed.

v5: wider fp8 coverage tuned to both the error budget and the chip's
power-state ceiling, plus LDWEIGHTS pair-sharing and ring balancing.
- 13 of 16 output row-groups run 3 fp8e4 DoubleRow k-pairs (contraction
  dims 0..767); the rest use 2 pairs. Host-side bit-exact simulation puts
  rel err at 1.934e-2 (sim matches HW to ~6 digits) under the 2e-2 gate.
- IMPORTANT: pushing to 14 KQ=3 groups (432 DR matmuls/core vs 424) trips
  a power-profile downclock -- the PE drops from 2.4 to ~2.0 GHz for the
  whole run (all matmuls 216 -> 259 ns) and the kernel LOSES ~80us. The
  DR-matmul density ceiling binds before the accuracy budget does.
- Down-projection and fp8 open matmuls of a chunk pair interleave so each
  256-col DR LDWEIGHTS serves two matmuls (halves DR weight-load stalls).
- Output is written bf16 (halves output HBM traffic; +3e-6 error var) and
  output DMAs alternate between the sync and scalar HWDGE rings; the last
  chunk's tail DMAs split across both rings.
- Startup: warm-tile memset on GpSimd, warmup+filler matmuls bridge the
  chunk-0 DMA ramp so HAM never re-throttles; chunk-1 loads are split so
  its down-projection starts on first-piece arrival.
"""

import numpy as np
import ml_dtypes

# Problem constants (hardcoded per harness contract).
N, D_IN, D_OUT, L, R = 32768, 2048, 2048, 8, 16
SCALE = 32.0 / 16.0
M_CORES = 8
NS = N // M_CORES  # 4096 tokens per core
P = 128
KT = D_IN // P  # 16 k-tiles
KQMAX = 3  # max fp8 DoubleRow k-PAIRS in the base matmul
N3 = 13  # output row-groups (of OI) using KQMAX pairs; the rest use 2
KB = KT - 4  # bf16 k-tile slots (k-tiles 4..15); KQ=3 groups skip the first 2
KD = KT // 2  # fp8 k-pairs in the down-projection (all 8)
OI = D_OUT // P  # 16 output row-chunks of 128
TW = 512  # token tile width (moving free dim)
TC = NS // TW  # 8 token chunks per core
LR = L * R  # 128
WG = 4  # W column groups
WGC = D_OUT // WG  # 512 columns per group
WARM = 112  # PE warmup matmuls before the first real matmul
FILL = 96  # startup filler matmuls (bridge chunk-0 DMA window)
FILL2 = 0  # seam fillers after quad-0: measured ~5us slower, keep 0
FILL3 = 0  # seam fillers before chunk-1: measured slower, keep 0
PAIR_ILV = True  # interleave chunk-pair DR matmuls (share LDWEIGHTS)
GATE_C1 = True  # gate chunk-1 loads behind quad-0 completion
GATE_LATE = True  # gate bF/g1..3 slabs behind the first down matmul
OUT_SPLIT = True  # alternate output DMAs between sync/scalar rings
OUT_BF16 = True  # write outT in bf16 (halves output HBM traffic)
TRIM_WT = True  # skip DMA of never-read bf16 weight slots
SWDGE_Q = 1  # GpSimd software-DGE queues
VEC_DRAIN = False  # vector-engine PSUM drain: measured no faster
SOLO_SPACED = False  # spaced solo-chunk opens: measured ~1us slower

SX = 32.0  # fp8 scale for x
SQ = 2048.0  # fp8 scale for W and A
GS = SX * SQ  # 2^16: global scale of all device-side math

_BF16 = ml_dtypes.bfloat16
_F8 = ml_dtypes.float8_e4m3

_CACHE = {}

LAST_EXEC_TIME_NS = None


def _kq_of(oi):
    return KQMAX if oi < N3 else 2


def _kb0_of(oi):
    # first usable bf16 k-tile slot (slot kb holds k-tile kb+4)
    return 2 if oi < N3 else 0


def _build():
    import concourse.bass as bass  # noqa: F401
    import concourse.tile as tile
    from concourse import bacc, mybir
    from concourse.tile_rust import add_dep_helper
    from contextlib import ExitStack

    bf16 = mybir.dt.bfloat16
    f8 = mybir.dt.float8e4
    f32 = mybir.dt.float32
    DR = mybir.MatmulPerfMode.DoubleRow

    nc = bacc.Bacc(
        "TRN2",
        target_bir_lowering=False,
        debug=False,
        num_devices=M_CORES,
        num_swdge_queues=SWDGE_Q,
    )

    # Host-prepared, partition-major layouts (see kernel()):
    #   xT [TC, P, KB, TW] bf16 : xT[t,p,kb,j] = x[t*TW+j, (kb+4)*P+p]
    #   xQ [TC, P, KD, 2, TW] f8: xQ[t,p,kk,u,j] = q8(x[t*TW+j, (2kk+u)*P+p]*SX)
    #   wT [WG, P, KB, WGC] bf16: wT[g,p,kb,o] = W[g*WGC+o, (kb+4)*P+p]*GS
    #   wQ [P, KQMAX, 2, D_OUT] f8: wQ[p,kk,u,o] = q8(W[o, (2kk+u)*P+p]*SQ)
    #   aQ [P, KD, 2, LR] f8    : aQ[p,kk,u,c] = q8(A_flat[c, (2kk+u)*P+p]*SQ)
    #   bF [P, D_OUT] bf16      : bF[c,o] = B_all[c//R, o, c%R]
    #   bias [P, OI] f32        : bias[p,oi] = b[oi*P+p]*GS
    #   mT [TC, P, TW] bf16     : one-hot adapter mask * SCALE
    xT = nc.dram_tensor("xT", [TC, P, KB, TW], bf16, kind="ExternalInput").ap()
    xQ = nc.dram_tensor("xQ", [TC, P, KD, 2, TW], f8, kind="ExternalInput").ap()
    wT = nc.dram_tensor("wT", [WG, P, KB, WGC], bf16, kind="ExternalInput").ap()
    wQ = nc.dram_tensor("wQ", [P, KQMAX, 2, D_OUT], f8, kind="ExternalInput").ap()
    aQ = nc.dram_tensor("aQ", [P, KD, 2, LR], f8, kind="ExternalInput").ap()
    bF = nc.dram_tensor("bF", [P, D_OUT], bf16, kind="ExternalInput").ap()
    bias = nc.dram_tensor("bias", [P, OI], f32, kind="ExternalInput").ap()
    mT = nc.dram_tensor("mT", [TC, P, TW], bf16, kind="ExternalInput").ap()
    out_dt = bf16 if OUT_BF16 else f32
    outT = nc.dram_tensor("outT", [D_OUT, NS], out_dt, kind="ExternalOutput").ap()

    with tile.TileContext(nc) as tc, ExitStack() as ctx:
        warm_pool = ctx.enter_context(tc.tile_pool(name="warm", bufs=1))
        aq_pool = ctx.enter_context(tc.tile_pool(name="aq", bufs=1))
        wq_pool = ctx.enter_context(tc.tile_pool(name="wq", bufs=1))
        bf_pool = ctx.enter_context(tc.tile_pool(name="bfp", bufs=1))
        bias_pool = ctx.enter_context(tc.tile_pool(name="bias", bufs=1))
        mask_pool = ctx.enter_context(tc.tile_pool(name="mask", bufs=4))
        # chunk-0 slab-loaded input tiles (stay resident; the w0 tile serves
        # as the g=0 weight tile for every chunk)
        xq0_pool = ctx.enter_context(tc.tile_pool(name="xq0", bufs=1))
        x0_pool = ctx.enter_context(tc.tile_pool(name="x0", bufs=1))
        w0_pool = ctx.enter_context(tc.tile_pool(name="w0", bufs=1))
        wt_pool = ctx.enter_context(tc.tile_pool(name="wt", bufs=WG - 1))
        xq_pool = ctx.enter_context(tc.tile_pool(name="xq", bufs=4))
        x_pool = ctx.enter_context(tc.tile_pool(name="x", bufs=4))
        u_pool = ctx.enter_context(tc.tile_pool(name="u", bufs=2))
        o_pool = ctx.enter_context(tc.tile_pool(name="o", bufs=4))
        pw_pool = ctx.enter_context(tc.tile_pool(name="pw", bufs=1, space="PSUM"))
        pu_pool = ctx.enter_context(tc.tile_pool(name="pu", bufs=2, space="PSUM"))
        po_pool = ctx.enter_context(tc.tile_pool(name="po", bufs=5, space="PSUM"))

        # Short PE warmup: covers the first small DMAs and starts the HAM
        # clock ramp while chunk-0 slices stream in.
        warm = warm_pool.tile([P, P], bf16)
        nc.gpsimd.memset(warm[:], 0.0)
        pw = pw_pool.tile([P, P], mybir.dt.float32)
        for _ in range(WARM):
            nc.tensor.matmul(pw[:], warm[:], warm[:], start=True, stop=True)

        def load_mask(t, gate=None):
            mk = mask_pool.tile([P, TW], bf16, tag="mk", name="mk")
            dma = nc.sync.dma_start(mk[:], mT[t])
            if gate is not None:
                add_dep_helper(dma.ins, gate.ins, sync=True, reason="pace")
            return mk

        # sync ring: small critical-path tensors, then chunk-0 slabs.
        aq = aq_pool.tile([P, KD, 2, LR], f8)
        nc.sync.dma_start(aq[:], aQ[:, :, :, :])
        xq0_t = xq0_pool.tile([P, KD, 2, TW], f8)
        for lo, hi in ((0, 3), (3, 6), (6, KD)):
            nc.sync.dma_start(xq0_t[:, lo:hi, :, :], xQ[0, :, lo:hi, :, :])
        mk0 = load_mask(0)
        bias_t = bias_pool.tile([P, OI], f32)
        nc.sync.dma_start(bias_t[:], bias[:, :])
        x0_t = x0_pool.tile([P, KB, TW], bf16)
        for lo in range(0, KB, 3):
            hi = min(lo + 3, KB)
            nc.sync.dma_start(x0_t[:, lo:hi, :], xT[0, :, lo:hi, :])

        # scalar ring: fp8 base weights, then g0 slabs (quad0 only needs
        # slots 2..11, so slot-2-first ordering feeds the kb-outer loop).
        wq = wq_pool.tile([P, KQMAX, 2, D_OUT], f8)
        nc.scalar.dma_start(wq[:], wQ[:, :, :, :])
        # group 0 serves only kq=3 rows (oi 0..3) -> slots 0,1 never read.
        w0_t = w0_pool.tile([P, KB, WGC], bf16)
        w0lo = 2 if (TRIM_WT and N3 >= 4) else 0
        for lo, hi in ((w0lo, 5), (5, 8), (8, KB)):
            nc.scalar.dma_start(w0_t[:, lo:hi, :], wT[0, :, lo:hi, :])
        bf_t = bf_pool.tile([P, D_OUT], bf16)
        bf_dma = nc.scalar.dma_start(bf_t[:], bF[:, :])
        # g1..g3 ride the idle GpSimd SWDGE queue: big transfers where the
        # ~2us software setup cost is amortized, keeping the two HWDGE rings
        # free for the latency-critical startup slabs. Their triggers (and
        # bF's) are gated behind the first down-projection matmul so the
        # chunk-0 critical slabs monopolize HBM bandwidth first.
        late_dmas = [bf_dma]
        wts = [w0_t]
        for g in range(1, WG):
            wt_g = wt_pool.tile([P, KB, WGC], bf16)
            # groups 1,2 serve only kq=3 rows -> slots 0,1 never read.
            lo = 2 if (TRIM_WT and N3 >= (g + 1) * 4) else 0
            late_dmas.append(
                nc.gpsimd.dma_start(wt_g[:, lo:, :], wT[g, :, lo:, :])
            )
            wts.append(wt_g)

        def load_chunk(t, gate=None, split=False):
            xq_c = xq_pool.tile([P, KD, 2, TW], f8)
            xb_c = x_pool.tile([P, KB, TW], bf16)
            dmas = []
            if split:
                for lo, hi in ((0, 3), (3, 6), (6, KD)):
                    dmas.append(
                        nc.sync.dma_start(xq_c[:, lo:hi, :, :], xQ[t, :, lo:hi, :, :])
                    )
                for lo in range(0, KB, 3):
                    hi = min(lo + 3, KB)
                    dmas.append(
                        nc.sync.dma_start(xb_c[:, lo:hi, :], xT[t, :, lo:hi, :])
                    )
            else:
                dmas.append(nc.sync.dma_start(xq_c[:], xQ[t]))
                dmas.append(nc.sync.dma_start(xb_c[:], xT[t]))
            if gate is not None:
                for d in dmas:
                    add_dep_helper(d.ins, gate.ins, sync=True, reason="pace")
            return (
                lambda kk, _x=xq_c: _x[:, kk, :, :],
                lambda kb, _x=xb_c: _x[:, kb, :],
            )

        def wslice(g, kb, loc):
            return wts[g][:, kb, loc : loc + P]

        def down(xq_slice, mk, after_first=None):
            """8 fp8 DoubleRow MMs + masked select; returns (um, first_mm)."""
            pu = pu_pool.tile([P, TW], mybir.dt.float32, tag="pu", name="pu")
            first = None
            for kk in range(KD):
                dmm = nc.tensor.matmul(
                    pu[:],
                    aq[:, kk, :, :],
                    xq_slice(kk),
                    start=(kk == 0),
                    stop=(kk == KD - 1),
                    perf_mode=DR,
                )
                if kk == 0:
                    first = dmm
                    if after_first is not None:
                        after_first()
            um = u_pool.tile([P, TW], bf16, tag="um", name="um")
            nc.vector.tensor_tensor(um[:], pu[:], mk[:], op=mybir.AluOpType.mult)
            return um, first

        def down_pair(xqa, xqb, mka, mkb):
            """Paired down-projection: each aq LDWEIGHTS serves 2 matmuls."""
            pu_a = pu_pool.tile([P, TW], mybir.dt.float32, tag="pu", name="pu")
            pu_b = pu_pool.tile([P, TW], mybir.dt.float32, tag="pu", name="pu")
            for kk in range(KD):
                st, sp = kk == 0, kk == KD - 1
                nc.tensor.matmul(
                    pu_a[:], aq[:, kk, :, :], xqa(kk), start=st, stop=sp,
                    perf_mode=DR,
                )
                nc.tensor.matmul(
                    pu_b[:], aq[:, kk, :, :], xqb(kk), start=st, stop=sp,
                    perf_mode=DR,
                )
            um_a = u_pool.tile([P, TW], bf16, tag="um", name="um")
            um_b = u_pool.tile([P, TW], bf16, tag="um", name="um")
            nc.vector.tensor_tensor(um_a[:], pu_a[:], mka[:], op=mybir.AluOpType.mult)
            nc.vector.tensor_tensor(um_b[:], pu_b[:], mkb[:], op=mybir.AluOpType.mult)
            return um_a, um_b

        def open_group(oi, xq_slice):
            # fp8 DoubleRow pairs cover contraction dims 0..256*kq-1.
            po = po_pool.tile([P, TW], mybir.dt.float32, tag="po", name="po")
            for kk in range(_kq_of(oi)):
                nc.tensor.matmul(
                    po[:],
                    wq[:, kk, :, oi * P : (oi + 1) * P],
                    xq_slice(kk),
                    start=(kk == 0),
                    stop=False,
                    perf_mode=DR,
                )
            return po

        def finish_group(t, oi, po, um, eng=None, split_out=False, veng=False):
            if not OUT_SPLIT:
                eng = None
                split_out = False
            up = nc.tensor.matmul(
                po[:],
                bf_t[:, oi * P : (oi + 1) * P],
                um[:],
                start=False,
                stop=True,
            )
            ot = o_pool.tile([P, TW], out_dt, tag="ot", name="ot")
            if veng and VEC_DRAIN:  # drain this member's PSUM on the DVE so
                # the two pair members' epilogues run on different engines
                nc.vector.tensor_scalar_add(ot[:], po[:], bias_t[:, oi : oi + 1])
            else:
                nc.scalar.add(ot[:], po[:], bias_t[:, oi : oi + 1])
            rows = slice(oi * P, (oi + 1) * P)
            if split_out:  # halve the final drain across both rings
                h = TW // 2
                c0 = t * TW
                nc.sync.dma_start(outT[rows, c0 : c0 + h], ot[:, 0:h])
                nc.scalar.dma_start(outT[rows, c0 + h : c0 + TW], ot[:, h:TW])
            else:
                (eng or nc.sync).dma_start(
                    outT[rows, t * TW : (t + 1) * TW], ot[:]
                )
            return up

        def solo_group(t, oi, xqs, xbs, um):
            # bf16-lead with spaced fp8 opens (same LDW pipelining as pairs)
            g = oi // WG
            loc = (oi % WG) * P
            kq = _kq_of(oi)
            kbs = list(range(_kb0_of(oi), KB))
            po = po_pool.tile([P, TW], mybir.dt.float32, tag="po", name="po")
            if SOLO_SPACED:
                nc.tensor.matmul(
                    po[:], wslice(g, kbs[0], loc), xbs(kbs[0]),
                    start=True, stop=False,
                )
                ki = 1
                for kk in range(kq):
                    nc.tensor.matmul(
                        po[:], wq[:, kk, :, oi * P : (oi + 1) * P], xqs(kk),
                        start=False, stop=False, perf_mode=DR,
                    )
                    if kk < kq - 1:
                        nc.tensor.matmul(
                            po[:], wslice(g, kbs[ki], loc), xbs(kbs[ki]),
                            start=False, stop=False,
                        )
                        ki += 1
                rest = kbs[ki:]
            else:
                for kk in range(kq):
                    nc.tensor.matmul(
                        po[:], wq[:, kk, :, oi * P : (oi + 1) * P], xqs(kk),
                        start=(kk == 0), stop=False, perf_mode=DR,
                    )
                rest = kbs
            for kb in rest:
                nc.tensor.matmul(
                    po[:], wslice(g, kb, loc), xbs(kb), start=False, stop=False
                )
            return finish_group(
                t, oi, po, um, eng=(nc.sync, nc.scalar)[oi % 2]
            )

        # ---- chunk 0 solo: kb-outer quad 0, then quads 1..3 ----
        xq0_slice = lambda kk: xq0_t[:, kk, :, :]  # noqa: E731
        xb0_slice = lambda kb: x0_t[:, kb, :]  # noqa: E731

        quad0_po = []

        def _open_quad0():
            # Interleave quad-0's fp8 group openers into the DMA-paced
            # down-projection window.
            for oi in range(4):
                quad0_po.append(open_group(oi, xq0_slice))

        um0, down0_first = down(xq0_slice, mk0, after_first=_open_quad0)
        if GATE_LATE:
            for dma in late_dmas:
                add_dep_helper(dma.ins, down0_first.ins, sync=True, reason="pace")

        # Filler matmuls keep the PE busy (and HAM warm) while the chunk-0
        # bf16 slabs stream in; they run only if quad-0 data hasn't landed.
        for _ in range(FILL):
            nc.tensor.matmul(pw[:], warm[:], warm[:], start=True, stop=True)

        # quad 0 @ t0: kb-outer so the PE consumes each k-slice on arrival.
        # (quad-0 groups all have kq=3 -> bf16 slots 2..11.)
        for kb in range(_kb0_of(0), KB):
            for oi in range(4):
                nc.tensor.matmul(
                    quad0_po[oi],
                    wslice(0, kb, oi * P),
                    xb0_slice(kb),
                    start=False,
                    stop=False,
                )
        quad0_last_up = None
        for oi in range(4):
            quad0_last_up = finish_group(
                0, oi, quad0_po[oi], um0, eng=(nc.sync, nc.scalar)[oi % 2]
            )
        # bridge the wait for the g1 weight slab so HAM stays warm
        for _ in range(FILL2):
            nc.tensor.matmul(pw[:], warm[:], warm[:], start=True, stop=True)
        for oi in range(4, OI):
            solo_group(0, oi, xq0_slice, xb0_slice, um0)

        # ---- chunk 1 solo (loads gated until quad 0 is done so the W
        # groups get the early HBM bandwidth) ----
        c1_gate = quad0_last_up if GATE_C1 else None
        xq1_slice, xb1_slice = load_chunk(1, gate=c1_gate, split=True)
        mk1 = load_mask(1, gate=c1_gate)
        for _ in range(FILL3):  # bridge the gated chunk-1 load arrival
            nc.tensor.matmul(pw[:], warm[:], warm[:], start=True, stop=True)
        um1, down1_first = down(xq1_slice, mk1)
        for oi in range(OI):
            solo_group(1, oi, xq1_slice, xb1_slice, um1)

        # ---- steady-state pairs (2,3), (4,5), (6,7) ----
        for tp in range(2, TC, 2):
            gate = down1_first if tp == 2 else None
            xqa, xba = load_chunk(tp, gate=gate)
            xqb, xbb = load_chunk(tp + 1, gate=gate)
            mka = load_mask(tp, gate=gate)
            mkb = load_mask(tp + 1, gate=gate)
            if PAIR_ILV:
                um_a, um_b = down_pair(xqa, xqb, mka, mkb)
            else:
                um_a, _ = down(xqa, mka)
                um_b, _ = down(xqb, mkb)
            for oi in range(OI):
                g = oi // WG
                loc = (oi % WG) * P
                kq = _kq_of(oi)
                kbs = list(range(_kb0_of(oi), KB))
                po_a = po_pool.tile([P, TW], mybir.dt.float32, tag="po", name="po")
                po_b = po_pool.tile([P, TW], mybir.dt.float32, tag="po", name="po")

                def bf_pair(kb, st=False):
                    ws = wslice(g, kb, loc)
                    nc.tensor.matmul(po_a[:], ws, xba(kb), start=st, stop=False)
                    nc.tensor.matmul(po_b[:], ws, xbb(kb), start=st, stop=False)

                def dr_pair(kk):
                    ws = wq[:, kk, :, oi * P : (oi + 1) * P]
                    nc.tensor.matmul(
                        po_a[:], ws, xqa(kk), start=False, stop=False,
                        perf_mode=DR,
                    )
                    nc.tensor.matmul(
                        po_b[:], ws, xqb(kk), start=False, stop=False,
                        perf_mode=DR,
                    )

                # bf16 leads (start=True); each DR open then follows a full
                # bf16-pair window so its 256-col LDWEIGHTS never stalls.
                ki = 0
                bf_pair(kbs[ki], st=True)
                ki += 1
                for kk in range(kq):
                    dr_pair(kk)
                    if kk < kq - 1:
                        bf_pair(kbs[ki])
                        ki += 1
                for kb in kbs[ki:]:
                    bf_pair(kb)
                ra, rb = (nc.sync, nc.scalar) if oi % 2 == 0 else (nc.scalar, nc.sync)
                last = tp + 1 == TC and oi >= OI - 6
                finish_group(tp, oi, po_a, um_a, eng=ra)
                finish_group(
                    tp + 1, oi, po_b, um_b, eng=rb, split_out=last, veng=last
                )

    nc.compile()
    return nc


def _get_nc():
    if "nc" not in _CACHE:
        _CACHE["nc"] = _build()
    return _CACHE["nc"]


def _install_trace_shim():
    """This image's antenv lacks axon_hooks; register the NTFF profile hook
    ourselves so run_bass_kernel_spmd(trace=True) can capture exec_time_ns."""
    import sys
    import types

    if "antenv.axon_hooks" in sys.modules:
        return
    import antenv

    mod = types.ModuleType("antenv.axon_hooks")
    state = {"hook": None}
    mod.set_axon_ntff_profile_hook = lambda h: state.__setitem__("hook", h)
    mod.get_axon_ntff_profile_hook = lambda: state["hook"]
    sys.modules["antenv.axon_hooks"] = mod
    antenv.axon_hooks = mod

    from trn_agent_boot.trn_boot import _ntff_profile_via_ctypes

    mod.set_axon_ntff_profile_hook(
        _ntff_profile_via_ctypes("/opt/axon/libaxon_pjrt.so")
    )

    # No S3 in this container; keep artifacts local.
    import concourse.bass_utils as bu

    bu.upload_artifacts = lambda tmpdir: f"local://{tmpdir}"


def _q8(a):
    return np.clip(a, -240.0, 240.0).astype(_F8)


def kernel(x, W, b, A_all, B_all, lora_idx, _trace=False):
    global LAST_EXEC_TIME_NS
    from concourse.bass_utils import run_bass_kernel_spmd

    if _trace:
        try:
            _install_trace_shim()
        except Exception as e:  # degrade to untraced run
            print(f"trace shim failed ({e!r}); running untraced")
            _trace = False

    x = np.asarray(x, dtype=np.float32)
    W = np.asarray(W, dtype=np.float32)
    b = np.asarray(b, dtype=np.float32)
    A_all = np.asarray(A_all, dtype=np.float32)
    B_all = np.asarray(B_all, dtype=np.float32)
    lora_idx = np.asarray(lora_idx, dtype=np.int32)

    # Host-side weight reformat (replicated across cores), partition-major.
    w4 = W.reshape(WG, WGC, KT, P).transpose(0, 3, 2, 1)  # [g,p,k,o]
    wT_np = np.ascontiguousarray((w4[:, :, 4:, :] * GS).astype(_BF16))
    wQ_np = np.ascontiguousarray(
        _q8(W[:, : 2 * KQMAX * P].reshape(D_OUT, KQMAX, 2, P) * SQ).transpose(
            3, 1, 2, 0
        )
    )
    A_flat = A_all.reshape(LR, D_IN)
    aQ_np = np.ascontiguousarray(
        _q8(A_flat.reshape(LR, KD, 2, P) * SQ).transpose(3, 1, 2, 0)
    )
    bF_np = np.ascontiguousarray(B_all.transpose(0, 2, 1)).reshape(LR, D_OUT).astype(
        _BF16
    )
    bias_np = np.ascontiguousarray((b * GS).reshape(OI, P).T).astype(np.float32)

    adapters = (np.arange(LR, dtype=np.int32) // R)[:, None]  # [LR, 1]

    in_maps = []
    for i in range(M_CORES):
        s = slice(i * NS, (i + 1) * NS)
        xr = x[s].reshape(TC, TW, KT, P)
        xT_i = np.ascontiguousarray(
            xr[:, :, 4:, :].astype(_BF16).transpose(0, 3, 2, 1)
        )
        xQ_i = np.ascontiguousarray(
            _q8(xr.reshape(TC, TW, KD, 2, P) * SX).transpose(0, 4, 2, 3, 1)
        )
        idx = lora_idx[s]
        mfull = (adapters == idx[None, :]).astype(np.float32) * SCALE  # [LR, NS]
        mT_i = np.ascontiguousarray(
            mfull.astype(_BF16).reshape(LR, TC, TW).transpose(1, 0, 2)
        )
        in_maps.append(
            {
                "xT": xT_i,
                "xQ": xQ_i,
                "wT": wT_np,
                "wQ": wQ_np,
                "aQ": aQ_np,
                "bF": bF_np,
                "bias": bias_np,
                "mT": mT_i,
            }
        )

    nc = _get_nc()
    res = run_bass_kernel_spmd(
        nc, in_maps, core_ids=list(range(M_CORES)), trace=_trace
    )
    LAST_EXEC_TIME_NS = res.exec_time_ns

    out = np.empty((N, D_OUT), dtype=np.float32)
    inv = np.float32(1.0 / GS)
    for i in range(M_CORES):
        r = res.results[i]["outT"]
        out[i * NS : (i + 1) * NS] = r.T.astype(np.float32) * inv
    return out
